# revision 30
# baseline (speedup 1.0000x reference)
"""ClassAttention Trainium2 kernel (Bass/Tile), data-parallel over batch on 8 cores.

Math (per batch b):
  q = x[b,0] @ W_q                      -> [H, D]
  k = x[b] @ W_k ; v = x[b] @ W_v       (W_k/W_v = halves of W_kv)
  scores = (q * SCALE) . k  per head    -> [H, N]
  attn = softmax(scores, axis=N)
  cls = attn @ v (per head)             -> [H*D]
  out[b] = cls @ W_proj + b_proj

Algebraic tricks eliminate both giant matmuls (x@W_k and x@W_v):
 1. Fold q into the weights so k is never materialized:
      Q'_b[64h+d, h] = q_b[h,d] * SCALE   (block-diagonal scatter, [C, H])
      G_b = W_k @ Q'_b                    ([C, H], per batch)
      scores^T = G_b^T @ x_b^T            (16-row x 512-col matmuls)
 2. Reassociate the value path: cls = (attn @ x) @ W_v
      y_b = attn_b @ x_b                  ([H, C], contraction over tokens,
                                           attn stationary, x natural moving)
      cls  = diag-blocks of (W_v^T y^T)   (one 128-col matmul for all batches)

v5 changes vs v3:
 - All inputs cast to bf16 on the host (the kernel already computed in bf16
   via cast-during-DMA, so numerics are unchanged).
 - x^T comes straight from the host (x transposed + token-permuted to the
   on-chip n' = g*128+p <-> token 8p+g order), removing all 512 per-batch
   PE transpose matmuls + their LDWEIGHTS + PSUM->SBUF copies.
 - Weights/bias/out ride the sync HWDGE ring; x and x^T ride the gpsimd
   ring, so weight transfers overlap the x stream and neither stalls.
Per-core HBM traffic: 16MB x + 16MB x^T + 8MB weights = 40MB bf16.
All matmuls bf16 with fp32 accumulation (sel-combines f32).  8 batches/core;
no collectives.
"""

import numpy as np
from contextlib import ExitStack

B, N, C = 64, 1024, 1024
H, D = 16, 64
SCALE = D**-0.5
NCORES = 8
BL = B // NCORES  # batches per core
CCH = C // 128  # chunks over any 1024-dim
GT = N // 128  # token groups per batch

_BUILT = {}


def _build_module():
    import concourse.mybir as mybir
    import concourse.tile as tile
    from concourse import bacc
    from concourse.masks import make_identity

    f32 = mybir.dt.float32
    bf16 = mybir.dt.bfloat16
    AF = mybir.ActivationFunctionType

    nc = bacc.Bacc("TRN2", target_bir_lowering=False, debug=False)

    # xT and weights come pre-arranged from the host in the exact SBUF
    # partition layout ([p, cc, ...] flattened) so every DMA moves 16KB
    # contiguous per partition (128 descriptors instead of 1024)
    x_d = nc.dram_tensor("x", [BL, N, C], bf16, kind="ExternalInput")
    xt_d = nc.dram_tensor("xT", [BL, 128, 2, CCH * 512], bf16, kind="ExternalInput")
    sel_d = nc.dram_tensor("sel4", [128, H], f32, kind="ExternalInput")
    wkt_d = nc.dram_tensor("W_kT", [128, CCH * C], bf16, kind="ExternalInput")
    wv_d = nc.dram_tensor("W_v", [128, CCH * H * D], bf16, kind="ExternalInput")
    wq_d = nc.dram_tensor("W_q", [128, CCH * H * D], bf16, kind="ExternalInput")
    wp_d = nc.dram_tensor("W_proj", [128, CCH * C], bf16, kind="ExternalInput")
    bp_d = nc.dram_tensor("b_proj", [C], f32, kind="ExternalInput")
    out_d = nc.dram_tensor("out", [BL, C], f32, kind="ExternalOutput")

    with tile.TileContext(nc) as tc, ExitStack() as ctx:
        const = ctx.enter_context(tc.tile_pool(name="const", bufs=1))
        work = ctx.enter_context(tc.tile_pool(name="work", bufs=2))
        xpool = ctx.enter_context(tc.tile_pool(name="xp", bufs=3))
        xtpool = ctx.enter_context(tc.tile_pool(name="xtp", bufs=3))
        apool = ctx.enter_context(tc.tile_pool(name="ap", bufs=5))
        ps_t = ctx.enter_context(tc.tile_pool(name="ps_t", bufs=2, space="PSUM"))
        ps_pp = ctx.enter_context(tc.tile_pool(name="ps_pp", bufs=1, space="PSUM"))
        ps_sc = ctx.enter_context(tc.tile_pool(name="ps_sc", bufs=2, space="PSUM"))

        # ---------------- identities ----------------
        ident_bf = const.tile([128, 128], bf16, tag="ident_bf")
        make_identity(nc, ident_bf[:, :])
        ident_f32 = const.tile([128, 128], f32, tag="ident_f32")
        make_identity(nc, ident_f32[:, :])

        # ---------------- persistent PSUM banks ----------------
        # scores use S0/S1 (halves), value path V0/V1; memset once so the
        # sel-combine never reads uninitialized partitions.
        ps_s0 = ps_pp.tile([128, 512], f32, tag="ppS0")
        ps_s1 = ps_pp.tile([128, 512], f32, tag="ppS1")
        ps_v0 = ps_pp.tile([128, 512], f32, tag="ppV0")
        ps_v1 = ps_pp.tile([128, 512], f32, tag="ppV1")
        ps_s = [ps_s0, ps_s1]
        ps_v = [ps_v0, ps_v1]
        nc.vector.memset(ps_s[0][:, :], 0.0)
        nc.vector.memset(ps_s[1][:, :], 0.0)
        nc.vector.memset(ps_v[0][:, :], 0.0)
        nc.vector.memset(ps_v[1][:, :], 0.0)

        # CLS-token rows (natural)
        xcls_nat = const.tile([BL, C], bf16, tag="xcls_nat")
        nc.gpsimd.dma_start(out=xcls_nat[:, :], in_=x_d[:, 0, :])
        # 4-way partition-group combiner: sel4[32j+h, h] = 1
        sel_sb = const.tile([128, H], f32, tag="sel4")
        nc.sync.dma_start(out=sel_sb[:, :], in_=sel_d[:, :])

        # ---------------- scores-path weights on the sync ring ----------------
        # (needed first; the sync ring carries only these 4MB so they land
        # early while the x stream owns the gpsimd ring)
        wq_sb = xtpool.tile([128, CCH, 1024], bf16, tag="xt")  # staged, recycled
        nc.sync.dma_start(out=wq_sb[:, :, :], in_=wq_d[:, :])
        wkT = const.tile([128, CCH, 1024], bf16, tag="wkT")  # [p(j), jc, c]
        nc.sync.dma_start(out=wkT[:, :, :], in_=wkt_d[:, :])
        b_bc = const.tile([BL, C], f32, tag="b_bc")  # bias broadcast to BL rows
        for r in range(BL):
            nc.sync.dma_start(out=b_bc[r : r + 1, :], in_=bp_d[:])
        # tail weights ride the gpsimd ring AFTER the last x batch (they are
        # only needed by cls/proj at the very end)
        wv_sb = const.tile([128, CCH, 1024], bf16, tag="wv")  # [p(c), cc, j]
        wp_sb = const.tile([128, CCH, 1024], bf16, tag="wp")  # [p(c'), cc, o]

        def load_tail_weights():
            nc.gpsimd.dma_start(out=wv_sb[:, :, :], in_=wv_d[:, :])
            nc.gpsimd.dma_start(out=wp_sb[:, :, :], in_=wp_d[:, :])

        # ---------------- x loads on the gpsimd ring ----------------
        # natural: token 8p+g at [p, g];  transposed: host pre-permuted so
        # xt[c', cc, n'] = x[token 8*(n'%128) + n'//128, cc*128+c']
        def load_x(b):
            xt = xtpool.tile([128, 2, CCH, 512], bf16, tag="xt")
            nc.sync.dma_start(out=xt[:, 0, :, :], in_=xt_d[b, :, 0, :])
            nc.sync.dma_start(out=xt[:, 1, :, :], in_=xt_d[b, :, 1, :])
            x_sb = xpool.tile([128, GT, C], bf16, tag="x")
            nc.gpsimd.dma_start(
                out=x_sb[:, :, :],
                in_=x_d[b, :, :].rearrange("(p g) c -> p g c", g=GT),
            )
            return x_sb, xt

        x_tiles = {0: load_x(0), 1: load_x(1)}

        # ---------------- xcls^T via PE transpose ----------------
        xclsT = const.tile([128, CCH, BL], bf16, tag="xclsT")  # [p(c), cc, b]
        for cc in range(CCH):
            ps_x = ps_t.tile([128, BL], f32, tag="ps_tr")
            nc.tensor.matmul(
                ps_x[:, :],
                xcls_nat[:, cc * 128 : (cc + 1) * 128],
                ident_bf[0:BL, 0:BL],
            )
            nc.vector.tensor_copy(xclsT[:, cc, :], ps_x[:, :])

        # ---------------- q for all batches (wide form) ----------------
        qn = work.tile([BL, C], f32, tag="qn")
        for half in range(2):
            psq = ps_s[half][0:BL, :]
            for cc in range(CCH):
                nc.tensor.matmul(
                    psq,
                    xclsT[:, cc, :],
                    wq_sb[:, cc, half * 512 : (half + 1) * 512],
                    start=(cc == 0),
                    stop=(cc == CCH - 1),
                )
            nc.vector.tensor_copy(qn[:, half * 512 : (half + 1) * 512], psq)

        # scatter q into block-diagonal Q' (SCALE folded): Q'[p(j), jc, b*H+h]
        qp_sb = const.tile([128, CCH, BL * H], bf16, tag="qp")
        nc.vector.memset(qp_sb[:, :, :], 0.0)
        for m in range(CCH):
            psqt = ps_t.tile([128, BL], f32, tag="ps_tr")
            nc.tensor.matmul(
                psqt[:, :], qn[:, m * 128 : (m + 1) * 128], ident_f32[0:BL, 0:BL]
            )
            # head of c' = 128*m + p is 2m + p//64
            qv = qp_sb[:, m, :].rearrange("p (b h) -> p h b", h=H)
            nc.scalar.activation(qv[0:64, 2 * m, :], psqt[0:64, :], AF.Copy, scale=SCALE)
            nc.scalar.activation(
                qv[64:128, 2 * m + 1, :], psqt[64:128, :], AF.Copy, scale=SCALE
            )

        # ---------------- G = W_k @ Q' (all batches) ----------------
        # computed wide as G^T = Q'^T W_k^T (qp stationary, 512-col moving),
        # then 8 PE transposes back to [p(c), cc, b*H+h]
        gT_sb = const.tile([128, C], bf16, tag="gT")  # [p(b*H+h), c]
        for half in range(2):
            ps_g = ps_s[half]
            for jc in range(CCH):
                nc.tensor.matmul(
                    ps_g[:, :],
                    qp_sb[:, jc, :],
                    wkT[:, jc, half * 512 : (half + 1) * 512],
                    start=(jc == 0),
                    stop=(jc == CCH - 1),
                )
            nc.vector.tensor_copy(gT_sb[:, half * 512 : (half + 1) * 512], ps_g[:, :])
        g_sb = const.tile([128, CCH, BL * H], bf16, tag="g")  # [p(c), cc, b*H+h]
        for cc in range(CCH):
            ps_gt = ps_t.tile([128, 128], f32, tag="ps_tr")
            nc.tensor.matmul(
                ps_gt[:, :], gT_sb[:, cc * 128 : (cc + 1) * 128], ident_bf[:, :]
            )
            if cc % 2 == 0:
                nc.vector.tensor_copy(g_sb[:, cc, :], ps_gt[:, :])
            else:
                nc.scalar.copy(g_sb[:, cc, :], ps_gt[:, :])

        # y^T for all batches: [p(c), cc, b*H+h]
        yT_all = const.tile([128, CCH, BL * H], bf16, tag="yT")
        out_all = const.tile([BL, C], f32, tag="out_all")

        # ---------------- per-batch stages (software-pipelined) ----------------
        # scores^T = G_b^T @ x^T : [H, N] accumulated in PSUM halves.
        # scores ~ N(0,1): exp directly from PSUM without max-subtraction.
        # attn stays UNNORMALIZED (bf16); 1/sum is folded into the yn copy.
        def stage_scores(b, xt):
            attnT = work.tile([H, N], bf16, tag="attnT")
            sume2 = work.tile([H, 2], f32, tag="sume")
            for half in range(2):
                # 4 concurrent 128x32-mode matmuls: group j accumulates its
                # 2 cc chunks at PSUM partitions 32j..32j+15
                ps_p = ps_s[half]
                for j in range(4):
                    for r in range(2):
                        cc = 2 * j + r
                        nc.tensor.matmul(
                            ps_p[32 * j : 32 * j + H, :],
                            g_sb[:, cc, b * H : (b + 1) * H],
                            xt[:, half, cc, :],
                            start=(r == 0),
                            stop=(r == 1),
                            tile_position=(0, 32 * j),
                        )
                sb_p = work.tile([128, 512], f32, tag="sb_part")
                nc.vector.tensor_copy(sb_p[:, :], ps_p[:, :])
                ps_x2 = ps_sc.tile([H, 512], f32, tag="ps_sc")
                nc.tensor.matmul(ps_x2[:, :], sel_sb[:, :], sb_p[:, :])
                nc.scalar.activation(
                    attnT[:, half * 512 : (half + 1) * 512],
                    ps_x2[:, :],
                    AF.Exp,
                    accum_out=sume2[:, half : half + 1],
                )
            sume = work.tile([H, 1], f32, tag="sume1")
            nc.vector.tensor_add(sume[:, :], sume2[:, 0:1], sume2[:, 1:2])
            rs = work.tile([H, 1], f32, tag="rs")
            nc.vector.reciprocal(rs[:, :], sume[:, :])
            return attnT, rs

        # attn tiles per score-group g (partition p <-> token 8p+g),
        # four PE transposes packed per PSUM tile -> one copy per quad
        def stage_attn_t(attnT):
            attn_tiles = []
            atv = attnT[:, :].rearrange("h (g p) -> h g p", p=128)
            for gp in range(GT // 4):
                ps_a = ps_t.tile([128, 4 * H], f32, tag="ps_tr")
                for i in range(4):
                    nc.tensor.matmul(
                        ps_a[:, i * H : (i + 1) * H],
                        atv[:, 4 * gp + i, :],
                        ident_bf[0:H, 0:H],
                    )
                a_sb = apool.tile([128, 4, H], bf16, tag="attn")
                if gp % 2 == 0:
                    nc.vector.tensor_copy(
                        a_sb[:, :, :], ps_a[:, :].rearrange("p (i h) -> p i h", i=4)
                    )
                else:
                    nc.scalar.copy(
                        a_sb[:, :, :], ps_a[:, :].rearrange("p (i h) -> p i h", i=4)
                    )
                attn_tiles.extend(a_sb[:, i, :] for i in range(4))
            return attn_tiles

        # y_b = attn_b @ x_b (natural form, attn stationary): [H, C] bf16,
        # the softmax 1/sum applied per-partition (per-head) in the copy
        def stage_value(x_sb, attn_tiles, rs):
            yn = work.tile([H, C], bf16, tag="yn")
            for half in range(2):
                ps_p = ps_v[half]
                for j in range(4):
                    for r in range(2):
                        g = 2 * j + r
                        nc.tensor.matmul(
                            ps_p[32 * j : 32 * j + H, :],
                            attn_tiles[g],
                            x_sb[:, g, half * 512 : (half + 1) * 512],
                            start=(r == 0),
                            stop=(r == 1),
                            tile_position=(0, 32 * j),
                        )
                sb_p = work.tile([128, 512], f32, tag="sb_part")
                nc.vector.tensor_copy(sb_p[:, :], ps_p[:, :])
                ps_y = ps_sc.tile([H, 512], f32, tag="ps_sc")
                nc.tensor.matmul(ps_y[:, :], sel_sb[:, :], sb_p[:, :])
                nc.scalar.activation(
                    yn[:, half * 512 : (half + 1) * 512],
                    ps_y[:, :],
                    AF.Copy,
                    scale=rs[:, :],
                )
            return yn

        # transpose y into yT_all[:, cc, b*H:(b+1)*H], 4 chunks per copy
        def stage_yt(b, yn):
            for cp in range(CCH // 4):
                ps_yt = ps_t.tile([128, 4 * H], f32, tag="ps_tr")
                for i in range(4):
                    cc = 4 * cp + i
                    nc.tensor.matmul(
                        ps_yt[:, i * H : (i + 1) * H],
                        yn[:, cc * 128 : (cc + 1) * 128],
                        ident_bf[0:H, 0:H],
                    )
                if cp % 2 == 0:
                    nc.vector.tensor_copy(
                        yT_all[:, 4 * cp : 4 * cp + 4, b * H : (b + 1) * H],
                        ps_yt[:, :].rearrange("p (i h) -> p i h", i=4),
                    )
                else:
                    nc.scalar.copy(
                        yT_all[:, 4 * cp : 4 * cp + 4, b * H : (b + 1) * H],
                        ps_yt[:, :].rearrange("p (i h) -> p i h", i=4),
                    )

        # ---------------- main loop, one-batch software pipeline ----------------
        # PE program order per iteration: scores(b+1) -> attnT(b) -> yT(b-1)
        # -> value(b), so batch b's softmax/copy latencies hide under batch
        # b+1's score matmuls and the PE never idles long enough to cool.
        sc = {0: stage_scores(0, x_tiles[0][1])}
        prev_yn = None
        for b in range(BL):
            x_sb, xt = x_tiles.pop(b)
            if b + 2 < BL:
                x_tiles[b + 2] = load_x(b + 2)
            if b == BL - 1:
                load_tail_weights()
            if b + 1 < BL:
                sc[b + 1] = stage_scores(b + 1, x_tiles[b + 1][1])
            attnT, rs = sc.pop(b)
            attn_tiles = stage_attn_t(attnT)
            if prev_yn is not None:
                stage_yt(b - 1, prev_yn)
            prev_yn = stage_value(x_sb, attn_tiles, rs)
        stage_yt(BL - 1, prev_yn)

        # ---------------- cls for all batches: diag blocks of W_v^T @ y^T ----
        # computed wide as cls-natural = (y^T)^T W_v (yT stationary, 512-col
        # moving), then 8 PE transposes + per-head diagonal extraction
        clsn = const.tile([128, C], bf16, tag="clsn")  # [p(b*H+h), c']
        clsT = const.tile([128, CCH, BL], bf16, tag="clsT")  # [p(c'), m, b]

        def cls_half(half):
            ps_cn = ps_s[half]
            for cc in range(CCH):
                nc.tensor.matmul(
                    ps_cn[:, :],
                    yT_all[:, cc, :],
                    wv_sb[:, cc, half * 512 : (half + 1) * 512],
                    start=(cc == 0),
                    stop=(cc == CCH - 1),
                )
            nc.vector.tensor_copy(clsn[:, half * 512 : (half + 1) * 512], ps_cn[:, :])

        def clsT_quarter(ms):
            for m in ms:
                ps_ct = ps_t.tile([128, 128], f32, tag="ps_tr")
                nc.tensor.matmul(
                    ps_ct[:, :], clsn[:, m * 128 : (m + 1) * 128], ident_bf[:, :]
                )
                # head of c' = 128m + p is 2m + p//64: pick column b*H + head
                pv = ps_ct[:, :].rearrange("p (b h) -> p h b", h=H)
                nc.scalar.copy(clsT[0:64, m, :], pv[0:64, 2 * m, :])
                nc.scalar.copy(clsT[64:128, m, :], pv[64:128, 2 * m + 1, :])

        # interleave so the clsT transposes of half0 run while half1's
        # accumulation + copy are still in flight
        cls_half(0)
        cls_half(1)
        clsT_quarter(range(0, 4))
        clsT_quarter(range(4, CCH))

        # ---------------- projection + bias (wide form) ----------------
        for half in range(2):
            ps_o = ps_v[half][0:BL, :]
            for cc in range(CCH):
                nc.tensor.matmul(
                    ps_o,
                    clsT[:, cc, :],
                    wp_sb[:, cc, half * 512 : (half + 1) * 512],
                    start=(cc == 0),
                    stop=(cc == CCH - 1),
                )
            nc.vector.tensor_add(
                out_all[:, half * 512 : (half + 1) * 512],
                ps_o,
                b_bc[:, half * 512 : (half + 1) * 512],
            )

        nc.sync.dma_start(out=out_d[:, :], in_=out_all[:, :])

    nc.compile()
    return nc


def get_module():
    if "nc" not in _BUILT:
        _BUILT["nc"] = _build_module()
    return _BUILT["nc"]


def make_in_maps(x, W_kv, W_q, W_proj, b_proj):
    """Host-side shard + layout prep (reordering/slicing/transposes + bf16
    cast; the kernel previously applied the identical bf16 cast during DMA)."""
    import ml_dtypes

    bf16 = ml_dtypes.bfloat16

    def to_sbuf_layout(w):
        # [1024, F] -> [128, CCH*F]: row cc*128+p lands at [p, cc*F : (cc+1)*F]
        f = w.shape[1]
        return np.ascontiguousarray(
            w.reshape(CCH, 128, f).transpose(1, 0, 2).reshape(128, CCH * f)
        )

    x = np.asarray(x, dtype=np.float32)
    xbf = np.ascontiguousarray(x.astype(bf16))
    # x^T with on-chip column order n' = g*128 + p  <->  token 8p+g, then
    # pre-arranged to the SBUF partition layout [p(c'), cc, n']:
    # xT[b, c, n'] = x[b, 8*(n'%128) + n'//128, c]
    xT = (
        xbf.transpose(0, 2, 1).reshape(B, C, 128, GT).transpose(0, 1, 3, 2).reshape(B, C, N)
    )
    xT = xT.reshape(B, CCH, 128, N).transpose(0, 2, 1, 3)  # [B, p, cc, n']
    # split into n'-halves so scores half0 can start as soon as its half lands:
    # layout [B, p, half, cc, 512]
    xT = np.ascontiguousarray(
        xT.reshape(B, 128, CCH, 2, 512).transpose(0, 1, 3, 2, 4).reshape(B, 128, 2, CCH * 512)
    )
    W_kv = np.asarray(W_kv, dtype=np.float32)
    W_kT = to_sbuf_layout(np.ascontiguousarray(W_kv[:, : H * D].T).astype(bf16))
    W_v = to_sbuf_layout(W_kv[:, H * D :].astype(bf16))
    W_q = to_sbuf_layout(np.asarray(W_q, dtype=np.float32).astype(bf16))
    W_proj = to_sbuf_layout(np.asarray(W_proj, dtype=np.float32).astype(bf16))
    b_proj = np.ascontiguousarray(np.asarray(b_proj, dtype=np.float32))
    sel4 = np.zeros((128, H), dtype=np.float32)
    for j in range(4):
        for h in range(H):
            sel4[32 * j + h, h] = 1.0
    in_maps = []
    for core in range(NCORES):
        in_maps.append(
            {
                "x": xbf[core * BL : (core + 1) * BL],
                "xT": xT[core * BL : (core + 1) * BL],
                "W_kT": W_kT,
                "W_v": W_v,
                "W_q": W_q,
                "W_proj": W_proj,
                "b_proj": b_proj,
                "sel4": sel4,
            }
        )
    return in_maps


def kernel(x, W_kv, W_q, W_proj, b_proj):
    from concourse.bass_utils import run_bass_kernel_spmd

    nc = get_module()
    in_maps = make_in_maps(x, W_kv, W_q, W_proj, b_proj)
    res = run_bass_kernel_spmd(nc, in_maps, core_ids=list(range(NCORES)))
    outs = [res.results[core]["out"] for core in range(NCORES)]
    return np.concatenate(outs, axis=0).reshape(B, 1, C).astype(np.float32)


# revision 31
# speedup vs baseline: 1.1447x; 1.1447x over previous
"""ClassAttention Trainium2 kernel (Bass/Tile), data-parallel over batch on 8 cores.

Math (per batch b):
  q = x[b,0] @ W_q                      -> [H, D]
  k = x[b] @ W_k ; v = x[b] @ W_v       (W_k/W_v = halves of W_kv)
  scores = (q * SCALE) . k  per head    -> [H, N]
  attn = softmax(scores, axis=N)
  cls = attn @ v (per head)             -> [H*D]
  out[b] = cls @ W_proj + b_proj

Algebraic tricks eliminate both giant matmuls (x@W_k and x@W_v):
 1. Fold q into the weights so k is never materialized:
      Q'_b[64h+d, h] = q_b[h,d] * SCALE   (block-diagonal scatter, [C, H])
      G_b = W_k @ Q'_b                    ([C, H], per batch)
      scores^T = G_b^T @ x_b^T            (16-row x 512-col matmuls)
 2. Reassociate the value path: cls = (attn @ x) @ W_v
      y_b = attn_b @ x_b                  ([H, C], contraction over tokens,
                                           attn stationary, x natural moving)
      cls  = diag-blocks of (W_v^T y^T)   (one 128-col matmul for all batches)

v5 changes vs v3:
 - All inputs cast to bf16 on the host (the kernel already computed in bf16
   via cast-during-DMA, so numerics are unchanged).
 - x^T comes straight from the host (x transposed + token-permuted to the
   on-chip n' = g*128+p <-> token 8p+g order), removing all 512 per-batch
   PE transpose matmuls + their LDWEIGHTS + PSUM->SBUF copies.
 - Weights/bias/out ride the sync HWDGE ring; x and x^T ride the gpsimd
   ring, so weight transfers overlap the x stream and neither stalls.
Per-core HBM traffic: 16MB x + 16MB x^T + 8MB weights = 40MB bf16.
All matmuls bf16 with fp32 accumulation (sel-combines f32).  8 batches/core;
no collectives.
"""

import numpy as np
from contextlib import ExitStack

B, N, C = 64, 1024, 1024
H, D = 16, 64
SCALE = D**-0.5
NCORES = 8
BL = B // NCORES  # batches per core
CCH = C // 128  # chunks over any 1024-dim
GT = N // 128  # token groups per batch

_BUILT = {}


def _build_module():
    import concourse.mybir as mybir
    import concourse.tile as tile
    from concourse import bacc
    from concourse.masks import make_identity

    f32 = mybir.dt.float32
    bf16 = mybir.dt.bfloat16
    AF = mybir.ActivationFunctionType

    nc = bacc.Bacc("TRN2", target_bir_lowering=False, debug=False)

    # xT and weights come pre-arranged from the host in the exact SBUF
    # partition layout ([p, cc, ...] flattened) so every DMA moves 16KB
    # contiguous per partition (128 descriptors instead of 1024)
    x_d = nc.dram_tensor("x", [BL, N, C], bf16, kind="ExternalInput")
    xt_d = nc.dram_tensor("xT", [BL, 128, 2, CCH * 512], bf16, kind="ExternalInput")
    sel_d = nc.dram_tensor("sel4", [128, H], f32, kind="ExternalInput")
    wkt_d = nc.dram_tensor("W_kT", [128, CCH * C], bf16, kind="ExternalInput")
    wv_d = nc.dram_tensor("W_v", [128, CCH * H * D], bf16, kind="ExternalInput")
    wq_d = nc.dram_tensor("W_q", [128, CCH * H * D], bf16, kind="ExternalInput")
    wp_d = nc.dram_tensor("W_proj", [128, CCH * C], bf16, kind="ExternalInput")
    bp_d = nc.dram_tensor("b_proj", [C], f32, kind="ExternalInput")
    out_d = nc.dram_tensor("out", [BL, C], f32, kind="ExternalOutput")

    with tile.TileContext(nc) as tc, ExitStack() as ctx:
        const = ctx.enter_context(tc.tile_pool(name="const", bufs=1))
        work = ctx.enter_context(tc.tile_pool(name="work", bufs=2))
        xpool = ctx.enter_context(tc.tile_pool(name="xp", bufs=3))
        xtpool = ctx.enter_context(tc.tile_pool(name="xtp", bufs=3))
        apool = ctx.enter_context(tc.tile_pool(name="ap", bufs=5))
        ps_t = ctx.enter_context(tc.tile_pool(name="ps_t", bufs=2, space="PSUM"))
        ps_pp = ctx.enter_context(tc.tile_pool(name="ps_pp", bufs=1, space="PSUM"))
        ps_sc = ctx.enter_context(tc.tile_pool(name="ps_sc", bufs=2, space="PSUM"))

        # ---------------- identities ----------------
        ident_bf = const.tile([128, 128], bf16, tag="ident_bf")
        make_identity(nc, ident_bf[:, :])
        ident_f32 = const.tile([128, 128], f32, tag="ident_f32")
        make_identity(nc, ident_f32[:, :])

        # ---------------- persistent PSUM banks ----------------
        # scores use S0/S1 (halves), value path V0/V1; memset once so the
        # sel-combine never reads uninitialized partitions.
        ps_s0 = ps_pp.tile([128, 512], f32, tag="ppS0")
        ps_s1 = ps_pp.tile([128, 512], f32, tag="ppS1")
        ps_v0 = ps_pp.tile([128, 512], f32, tag="ppV0")
        ps_v1 = ps_pp.tile([128, 512], f32, tag="ppV1")
        ps_s = [ps_s0, ps_s1]
        ps_v = [ps_v0, ps_v1]
        nc.vector.memset(ps_s[0][:, :], 0.0)
        nc.vector.memset(ps_s[1][:, :], 0.0)
        nc.vector.memset(ps_v[0][:, :], 0.0)
        nc.vector.memset(ps_v[1][:, :], 0.0)

        # CLS-token rows (natural)
        xcls_nat = const.tile([BL, C], bf16, tag="xcls_nat")
        nc.gpsimd.dma_start(out=xcls_nat[:, :], in_=x_d[:, 0, :])
        # 4-way partition-group combiner: sel4[32j+h, h] = 1
        sel_sb = const.tile([128, H], f32, tag="sel4")
        nc.sync.dma_start(out=sel_sb[:, :], in_=sel_d[:, :])

        # ---------------- scores-path weights on the sync ring ----------------
        # (needed first; the sync ring carries only these 4MB so they land
        # early while the x stream owns the gpsimd ring)
        wq_sb = xtpool.tile([128, CCH, 1024], bf16, tag="xt")  # staged, recycled
        nc.sync.dma_start(out=wq_sb[:, :, :], in_=wq_d[:, :])
        wkT = const.tile([128, CCH, 1024], bf16, tag="wkT")  # [p(j), jc, c]
        nc.sync.dma_start(out=wkT[:, :, :], in_=wkt_d[:, :])
        b_bc = const.tile([BL, C], f32, tag="b_bc")  # bias broadcast to BL rows
        for r in range(BL):
            nc.sync.dma_start(out=b_bc[r : r + 1, :], in_=bp_d[:])
        # tail weights ride the gpsimd ring AFTER the last x batch (they are
        # only needed by cls/proj at the very end)
        wv_sb = const.tile([128, CCH, 1024], bf16, tag="wv")  # [p(c), cc, j]
        wp_sb = const.tile([128, CCH, 1024], bf16, tag="wp")  # [p(c'), cc, o]

        def load_tail_weights():
            nc.gpsimd.dma_start(out=wv_sb[:, :, :], in_=wv_d[:, :])
            nc.gpsimd.dma_start(out=wp_sb[:, :, :], in_=wp_d[:, :])

        # ---------------- x loads on the gpsimd ring ----------------
        # natural: token 8p+g at [p, g];  transposed: host pre-permuted so
        # xt[c', cc, n'] = x[token 8*(n'%128) + n'//128, cc*128+c']
        def load_x(b):
            xt = xtpool.tile([128, 2, CCH, 512], bf16, tag="xt")
            nc.gpsimd.dma_start(out=xt[:, 0, :, :], in_=xt_d[b, :, 0, :])
            nc.gpsimd.dma_start(out=xt[:, 1, :, :], in_=xt_d[b, :, 1, :])
            x_sb = xpool.tile([128, GT, C], bf16, tag="x")
            nc.gpsimd.dma_start(
                out=x_sb[:, :, :],
                in_=x_d[b, :, :].rearrange("(p g) c -> p g c", g=GT),
            )
            return x_sb, xt

        x_tiles = {0: load_x(0), 1: load_x(1)}

        # ---------------- xcls^T via PE transpose ----------------
        xclsT = const.tile([128, CCH, BL], bf16, tag="xclsT")  # [p(c), cc, b]
        for cc in range(CCH):
            ps_x = ps_t.tile([128, BL], f32, tag="ps_tr")
            nc.tensor.matmul(
                ps_x[:, :],
                xcls_nat[:, cc * 128 : (cc + 1) * 128],
                ident_bf[0:BL, 0:BL],
            )
            nc.vector.tensor_copy(xclsT[:, cc, :], ps_x[:, :])

        # ---------------- q for all batches (wide form) ----------------
        qn = work.tile([BL, C], f32, tag="qn")
        for half in range(2):
            psq = ps_s[half][0:BL, :]
            for cc in range(CCH):
                nc.tensor.matmul(
                    psq,
                    xclsT[:, cc, :],
                    wq_sb[:, cc, half * 512 : (half + 1) * 512],
                    start=(cc == 0),
                    stop=(cc == CCH - 1),
                )
            nc.vector.tensor_copy(qn[:, half * 512 : (half + 1) * 512], psq)

        # scatter q into block-diagonal Q' (SCALE folded): Q'[p(j), jc, b*H+h]
        qp_sb = const.tile([128, CCH, BL * H], bf16, tag="qp")
        nc.vector.memset(qp_sb[:, :, :], 0.0)
        for m in range(CCH):
            psqt = ps_t.tile([128, BL], f32, tag="ps_tr")
            nc.tensor.matmul(
                psqt[:, :], qn[:, m * 128 : (m + 1) * 128], ident_f32[0:BL, 0:BL]
            )
            # head of c' = 128*m + p is 2m + p//64
            qv = qp_sb[:, m, :].rearrange("p (b h) -> p h b", h=H)
            nc.scalar.activation(qv[0:64, 2 * m, :], psqt[0:64, :], AF.Copy, scale=SCALE)
            nc.scalar.activation(
                qv[64:128, 2 * m + 1, :], psqt[64:128, :], AF.Copy, scale=SCALE
            )

        # ---------------- G = W_k @ Q' (all batches) ----------------
        # computed wide as G^T = Q'^T W_k^T (qp stationary, 512-col moving),
        # then 8 PE transposes back to [p(c), cc, b*H+h]
        gT_sb = const.tile([128, C], bf16, tag="gT")  # [p(b*H+h), c]
        for half in range(2):
            ps_g = ps_s[half]
            for jc in range(CCH):
                nc.tensor.matmul(
                    ps_g[:, :],
                    qp_sb[:, jc, :],
                    wkT[:, jc, half * 512 : (half + 1) * 512],
                    start=(jc == 0),
                    stop=(jc == CCH - 1),
                )
            nc.vector.tensor_copy(gT_sb[:, half * 512 : (half + 1) * 512], ps_g[:, :])
        g_sb = const.tile([128, CCH, BL * H], bf16, tag="g")  # [p(c), cc, b*H+h]
        for cc in range(CCH):
            ps_gt = ps_t.tile([128, 128], f32, tag="ps_tr")
            nc.tensor.matmul(
                ps_gt[:, :], gT_sb[:, cc * 128 : (cc + 1) * 128], ident_bf[:, :]
            )
            if cc % 2 == 0:
                nc.vector.tensor_copy(g_sb[:, cc, :], ps_gt[:, :])
            else:
                nc.scalar.copy(g_sb[:, cc, :], ps_gt[:, :])

        # y^T for all batches: [p(c), cc, b*H+h]
        yT_all = const.tile([128, CCH, BL * H], bf16, tag="yT")
        out_all = const.tile([BL, C], f32, tag="out_all")

        # ---------------- per-batch stages (software-pipelined) ----------------
        # scores^T = G_b^T @ x^T : [H, N] accumulated in PSUM halves.
        # scores ~ N(0,1): exp directly from PSUM without max-subtraction.
        # attn stays UNNORMALIZED (bf16); 1/sum is folded into the yn copy.
        def stage_scores(b, xt):
            attnT = work.tile([H, N], bf16, tag="attnT")
            sume2 = work.tile([H, 2], f32, tag="sume")
            for half in range(2):
                # 4 concurrent 128x32-mode matmuls: group j accumulates its
                # 2 cc chunks at PSUM partitions 32j..32j+15
                ps_p = ps_s[half]
                for j in range(4):
                    for r in range(2):
                        cc = 2 * j + r
                        nc.tensor.matmul(
                            ps_p[32 * j : 32 * j + H, :],
                            g_sb[:, cc, b * H : (b + 1) * H],
                            xt[:, half, cc, :],
                            start=(r == 0),
                            stop=(r == 1),
                            tile_position=(0, 32 * j),
                        )
                sb_p = work.tile([128, 512], f32, tag="sb_part")
                nc.vector.tensor_copy(sb_p[:, :], ps_p[:, :])
                ps_x2 = ps_sc.tile([H, 512], f32, tag="ps_sc")
                nc.tensor.matmul(ps_x2[:, :], sel_sb[:, :], sb_p[:, :])
                nc.scalar.activation(
                    attnT[:, half * 512 : (half + 1) * 512],
                    ps_x2[:, :],
                    AF.Exp,
                    accum_out=sume2[:, half : half + 1],
                )
            sume = work.tile([H, 1], f32, tag="sume1")
            nc.vector.tensor_add(sume[:, :], sume2[:, 0:1], sume2[:, 1:2])
            rs = work.tile([H, 1], f32, tag="rs")
            nc.vector.reciprocal(rs[:, :], sume[:, :])
            return attnT, rs

        # attn tiles per score-group g (partition p <-> token 8p+g),
        # four PE transposes packed per PSUM tile -> one copy per quad
        def stage_attn_t(attnT):
            attn_tiles = []
            atv = attnT[:, :].rearrange("h (g p) -> h g p", p=128)
            for gp in range(GT // 4):
                ps_a = ps_t.tile([128, 4 * H], f32, tag="ps_tr")
                for i in range(4):
                    nc.tensor.matmul(
                        ps_a[:, i * H : (i + 1) * H],
                        atv[:, 4 * gp + i, :],
                        ident_bf[0:H, 0:H],
                    )
                a_sb = apool.tile([128, 4, H], bf16, tag="attn")
                if gp % 2 == 0:
                    nc.vector.tensor_copy(
                        a_sb[:, :, :], ps_a[:, :].rearrange("p (i h) -> p i h", i=4)
                    )
                else:
                    nc.scalar.copy(
                        a_sb[:, :, :], ps_a[:, :].rearrange("p (i h) -> p i h", i=4)
                    )
                attn_tiles.extend(a_sb[:, i, :] for i in range(4))
            return attn_tiles

        # y_b = attn_b @ x_b (natural form, attn stationary): [H, C] bf16,
        # the softmax 1/sum applied per-partition (per-head) in the copy
        def stage_value(x_sb, attn_tiles, rs):
            yn = work.tile([H, C], bf16, tag="yn")
            for half in range(2):
                ps_p = ps_v[half]
                for j in range(4):
                    for r in range(2):
                        g = 2 * j + r
                        nc.tensor.matmul(
                            ps_p[32 * j : 32 * j + H, :],
                            attn_tiles[g],
                            x_sb[:, g, half * 512 : (half + 1) * 512],
                            start=(r == 0),
                            stop=(r == 1),
                            tile_position=(0, 32 * j),
                        )
                sb_p = work.tile([128, 512], f32, tag="sb_part")
                nc.vector.tensor_copy(sb_p[:, :], ps_p[:, :])
                ps_y = ps_sc.tile([H, 512], f32, tag="ps_sc")
                nc.tensor.matmul(ps_y[:, :], sel_sb[:, :], sb_p[:, :])
                nc.scalar.activation(
                    yn[:, half * 512 : (half + 1) * 512],
                    ps_y[:, :],
                    AF.Copy,
                    scale=rs[:, :],
                )
            return yn

        # transpose y into yT_all[:, cc, b*H:(b+1)*H], 4 chunks per copy
        def stage_yt(b, yn):
            for cp in range(CCH // 4):
                ps_yt = ps_t.tile([128, 4 * H], f32, tag="ps_tr")
                for i in range(4):
                    cc = 4 * cp + i
                    nc.tensor.matmul(
                        ps_yt[:, i * H : (i + 1) * H],
                        yn[:, cc * 128 : (cc + 1) * 128],
                        ident_bf[0:H, 0:H],
                    )
                if cp % 2 == 0:
                    nc.vector.tensor_copy(
                        yT_all[:, 4 * cp : 4 * cp + 4, b * H : (b + 1) * H],
                        ps_yt[:, :].rearrange("p (i h) -> p i h", i=4),
                    )
                else:
                    nc.scalar.copy(
                        yT_all[:, 4 * cp : 4 * cp + 4, b * H : (b + 1) * H],
                        ps_yt[:, :].rearrange("p (i h) -> p i h", i=4),
                    )

        # ---------------- main loop, one-batch software pipeline ----------------
        # PE program order per iteration: scores(b+1) -> attnT(b) -> yT(b-1)
        # -> value(b), so batch b's softmax/copy latencies hide under batch
        # b+1's score matmuls and the PE never idles long enough to cool.
        sc = {0: stage_scores(0, x_tiles[0][1])}
        prev_yn = None
        for b in range(BL):
            x_sb, xt = x_tiles.pop(b)
            if b + 2 < BL:
                x_tiles[b + 2] = load_x(b + 2)
            if b == BL - 1:
                load_tail_weights()
            if b + 1 < BL:
                sc[b + 1] = stage_scores(b + 1, x_tiles[b + 1][1])
            attnT, rs = sc.pop(b)
            attn_tiles = stage_attn_t(attnT)
            if prev_yn is not None:
                stage_yt(b - 1, prev_yn)
            prev_yn = stage_value(x_sb, attn_tiles, rs)
        stage_yt(BL - 1, prev_yn)

        # ---------------- cls for all batches: diag blocks of W_v^T @ y^T ----
        # computed wide as cls-natural = (y^T)^T W_v (yT stationary, 512-col
        # moving), then 8 PE transposes + per-head diagonal extraction
        clsn = const.tile([128, C], bf16, tag="clsn")  # [p(b*H+h), c']
        clsT = const.tile([128, CCH, BL], bf16, tag="clsT")  # [p(c'), m, b]

        def cls_half(half):
            ps_cn = ps_s[half]
            for cc in range(CCH):
                nc.tensor.matmul(
                    ps_cn[:, :],
                    yT_all[:, cc, :],
                    wv_sb[:, cc, half * 512 : (half + 1) * 512],
                    start=(cc == 0),
                    stop=(cc == CCH - 1),
                )
            nc.vector.tensor_copy(clsn[:, half * 512 : (half + 1) * 512], ps_cn[:, :])

        def clsT_quarter(ms):
            for m in ms:
                ps_ct = ps_t.tile([128, 128], f32, tag="ps_tr")
                nc.tensor.matmul(
                    ps_ct[:, :], clsn[:, m * 128 : (m + 1) * 128], ident_bf[:, :]
                )
                # head of c' = 128m + p is 2m + p//64: pick column b*H + head
                pv = ps_ct[:, :].rearrange("p (b h) -> p h b", h=H)
                nc.scalar.copy(clsT[0:64, m, :], pv[0:64, 2 * m, :])
                nc.scalar.copy(clsT[64:128, m, :], pv[64:128, 2 * m + 1, :])

        # interleave so the clsT transposes of half0 run while half1's
        # accumulation + copy are still in flight
        cls_half(0)
        cls_half(1)
        clsT_quarter(range(0, 4))
        clsT_quarter(range(4, CCH))

        # ---------------- projection + bias (wide form) ----------------
        for half in range(2):
            ps_o = ps_v[half][0:BL, :]
            for cc in range(CCH):
                nc.tensor.matmul(
                    ps_o,
                    clsT[:, cc, :],
                    wp_sb[:, cc, half * 512 : (half + 1) * 512],
                    start=(cc == 0),
                    stop=(cc == CCH - 1),
                )
            nc.vector.tensor_add(
                out_all[:, half * 512 : (half + 1) * 512],
                ps_o,
                b_bc[:, half * 512 : (half + 1) * 512],
            )

        nc.sync.dma_start(out=out_d[:, :], in_=out_all[:, :])

    nc.compile()
    return nc


def get_module():
    if "nc" not in _BUILT:
        _BUILT["nc"] = _build_module()
    return _BUILT["nc"]


def make_in_maps(x, W_kv, W_q, W_proj, b_proj):
    """Host-side shard + layout prep (reordering/slicing/transposes + bf16
    cast; the kernel previously applied the identical bf16 cast during DMA)."""
    import ml_dtypes

    bf16 = ml_dtypes.bfloat16

    def to_sbuf_layout(w):
        # [1024, F] -> [128, CCH*F]: row cc*128+p lands at [p, cc*F : (cc+1)*F]
        f = w.shape[1]
        return np.ascontiguousarray(
            w.reshape(CCH, 128, f).transpose(1, 0, 2).reshape(128, CCH * f)
        )

    x = np.asarray(x, dtype=np.float32)
    xbf = np.ascontiguousarray(x.astype(bf16))
    # x^T with on-chip column order n' = g*128 + p  <->  token 8p+g, then
    # pre-arranged to the SBUF partition layout [p(c'), cc, n']:
    # xT[b, c, n'] = x[b, 8*(n'%128) + n'//128, c]
    xT = (
        xbf.transpose(0, 2, 1).reshape(B, C, 128, GT).transpose(0, 1, 3, 2).reshape(B, C, N)
    )
    xT = xT.reshape(B, CCH, 128, N).transpose(0, 2, 1, 3)  # [B, p, cc, n']
    # split into n'-halves so scores half0 can start as soon as its half lands:
    # layout [B, p, half, cc, 512]
    xT = np.ascontiguousarray(
        xT.reshape(B, 128, CCH, 2, 512).transpose(0, 1, 3, 2, 4).reshape(B, 128, 2, CCH * 512)
    )
    W_kv = np.asarray(W_kv, dtype=np.float32)
    W_kT = to_sbuf_layout(np.ascontiguousarray(W_kv[:, : H * D].T).astype(bf16))
    W_v = to_sbuf_layout(W_kv[:, H * D :].astype(bf16))
    W_q = to_sbuf_layout(np.asarray(W_q, dtype=np.float32).astype(bf16))
    W_proj = to_sbuf_layout(np.asarray(W_proj, dtype=np.float32).astype(bf16))
    b_proj = np.ascontiguousarray(np.asarray(b_proj, dtype=np.float32))
    sel4 = np.zeros((128, H), dtype=np.float32)
    for j in range(4):
        for h in range(H):
            sel4[32 * j + h, h] = 1.0
    in_maps = []
    for core in range(NCORES):
        in_maps.append(
            {
                "x": xbf[core * BL : (core + 1) * BL],
                "xT": xT[core * BL : (core + 1) * BL],
                "W_kT": W_kT,
                "W_v": W_v,
                "W_q": W_q,
                "W_proj": W_proj,
                "b_proj": b_proj,
                "sel4": sel4,
            }
        )
    return in_maps


def kernel(x, W_kv, W_q, W_proj, b_proj):
    from concourse.bass_utils import run_bass_kernel_spmd

    nc = get_module()
    in_maps = make_in_maps(x, W_kv, W_q, W_proj, b_proj)
    res = run_bass_kernel_spmd(nc, in_maps, core_ids=list(range(NCORES)))
    outs = [res.results[core]["out"] for core in range(NCORES)]
    return np.concatenate(outs, axis=0).reshape(B, 1, C).astype(np.float32)
